# revision 13
# baseline (speedup 1.0000x reference)
"""LSTM decoder (nn_Decoder) on 8 trn2 NeuronCores.

Tensor-parallel over the gate dimension with a remote-DMA-broadcast
all-gather. Each core owns a 128-row slice of h/c and its 512 gate rows
(i,f,g,o x 128). The reference feeds the LSTM output back as both input
and hidden state (x_t = h_t), so steps >= 2 use h @ (w_ih + w_hh).T + b;
step 1 (x0 = 0) uses w_hh alone.

Exchange: each core fires ONE remote_dma_broadcast per step that writes
its hT [128, 64] bf16 tile directly into slot <rank> of every core's
SBUF gather buffer (dynamic out_ap offset = partition_id * 64), bumping
a remote semaphore (+2/sender, +16/round). No DRAM bounce, no ncfw
collective. 4 gather buffers + 2 parity semaphores give the
out-projection (riding one step behind on the PE) a safe read window.

Per step, the gates PSUM is split into region A = {i,f,g} (cols 0:384)
and B = {o} (384:512), each its own accumulation group, so the ACT
sigmoid/tanh start after A's 8 k-matmuls instead of all 16. The h tile
is produced in transposed layout: PE transposes o and tanh(c) into
PSUM, DVE multiplies them straight into the bf16 hT send tile (no
PSUM->SBUF copy on the critical path). Bias matmuls are prefetched into
PSUM during the exchange flight; lsem-gated dummy matmuls keep the PE
pstate high through the flight window.

Semaphore bookkeeping (step t = 1..S):
  mma:  h0 = 1, gates_t region-A stop = t+1.
  mmb:  gates_t region-B stop = t.
  actsem: sig_if_t=5t-4, tanh_g_t=5t-3, sig_o_t=5t-2, tanh_c_t=5t-1,
          oTcopy_t=5t.
  dvesem: t=1: c_1=1, hT_1=2;  t>=2: c2=4t-5, t1=4t-4, c=4t-3, hT=4t-2.
  ptsem: transpose_o_t=2t-1, transpose_tanhc_t=2t.
  hsem: 1 after the h0 tile copy (exchange 0's payload).
  osem/ocp: outproj psum group / out_acc copy for step j -> j.
  lsem: +16 per broadcast send-complete; psem: +1 per prep.
"""
import os
import sys

sys.path.insert(0, "/opt/trn_rl_repo")

import numpy as np
import ml_dtypes

BF16 = ml_dtypes.bfloat16

B = 64          # batch
L = 256         # latent dim
H = 1024        # hidden
O = 512         # output dim
S = 256         # seq len
NC = 8          # cores
HL = H // NC    # 128, per-core h slice
GS = 4 * HL     # 512, per-core gate rows
OL = O // NC    # 64, per-core out slice
GA = 3 * HL     # 384, region A cols (i, f, g)


def ACT_SIF(t):
    return 5 * t - 4


def ACT_TG(t):
    return 5 * t - 3


def ACT_SO(t):
    return 5 * t - 2


def ACT_TC(t):
    return 5 * t - 1


def ACT_OC(t):
    return 5 * t


def DVE_C(t):
    return 1 if t == 1 else 4 * t - 3


def DVE_H(t):
    return 2 if t == 1 else 4 * t - 2


def _build_nc(s_len, dummies_mid=3, dummies_send=4, dummies_flight=12,
              dummies_pre=4, mode="full"):
    from concourse import bass, mybir
    from concourse import bacc
    from concourse.ap import AP
    from contextlib import ExitStack

    S_ = s_len
    nc = bacc.Bacc("TRN2", debug=False)
    f32 = mybir.dt.float32
    bf16 = mybir.dt.bfloat16
    AF = mybir.ActivationFunctionType
    ALU = mybir.AluOpType

    d_lat = nc.dram_tensor("latT", [128, 2 * B], bf16, kind="ExternalInput")
    d_fcw = nc.dram_tensor("fcwT", [128, 2 * HL], bf16, kind="ExternalInput")
    d_fcb = nc.dram_tensor("fcb", [128, 1], f32, kind="ExternalInput")
    d_wc = nc.dram_tensor("wcT", [128, NC * GS], bf16, kind="ExternalInput")
    d_whh = nc.dram_tensor("whhT", [128, NC * GS], bf16, kind="ExternalInput")
    d_outw = nc.dram_tensor("outwT", [128, NC * OL], bf16, kind="ExternalInput")
    d_bias = nc.dram_tensor("biasc", [1, GS], bf16, kind="ExternalInput")
    d_misc = nc.dram_tensor("misc", [B, 3 * B], bf16, kind="ExternalInput")
    d_identf = nc.dram_tensor("identf", [B, B], f32, kind="ExternalInput")
    d_out = nc.dram_tensor("out", [B, S_ * OL], f32, kind="ExternalOutput")

    ctx = ExitStack()
    sem = lambda n: ctx.enter_context(nc.semaphore(n))
    sb = lambda n, sh, dt: ctx.enter_context(nc.sbuf_tensor(n, sh, dt))
    ps = lambda n, sh, dt: ctx.enter_context(nc.psum_tensor(n, sh, dt))

    in_dma = sem("in_dma")     # input loads, 9 x +16 = 144
    gsem = [sem("gsem0"), sem("gsem1")]   # remote arrivals, +16/round/parity
    lsem = sem("lsem")         # broadcast send-complete, +16 each
    psem = sem("psem")         # prep desc-gen done, +1 each
    mma = sem("mma")           # gates region-A group: h0 -> 1, gates_t -> t+1
    mmb = sem("mmb")           # gates region-B group -> t
    osem = sem("osem")         # outproj psum group done -> t
    ocp = sem("ocp")           # outproj copied to out_acc -> t
    actsem = sem("actsem")     # ACT ops, see module docstring
    dvesem = sem("dvesem")     # DVE ops, see module docstring
    hsem = sem("hsem")         # h0 tile ready
    ptsem = sem("ptsem")       # PE transposes -> 2t-1, 2t
    odma = sem("odma")         # out chunks

    lat_sb = sb("lat_sb", [128, 2 * B], bf16)
    fcw_sb = sb("fcw_sb", [128, 2 * HL], bf16)
    fcb_sb = sb("fcb_sb", [128, 1], f32)
    wc_sb = sb("wc_sb", [128, NC * GS], bf16)
    whh_sb = sb("whh_sb", [128, NC * GS], bf16)
    outw_sb = sb("outw_sb", [128, NC * OL], bf16)
    bias_sb = sb("bias_sb", [1, GS], bf16)
    misc_sb = sb("misc_sb", [B, 3 * B], bf16)
    identf_sb = sb("identf_sb", [B, B], f32)
    gather = [sb(f"gather{b_}", [128, NC * B], bf16) for b_ in range(4)]
    hT_sb = [sb("hT0", [128, B], bf16), sb("hT1", [128, B], bf16)]
    c_sb = sb("c_sb", [B, HL], f32)
    c2_sb = sb("c2_sb", [B, HL], f32)
    t1_sb = sb("t1_sb", [B, HL], f32)
    tanhc_sb = sb("tanhc_sb", [B, HL], f32)
    ifo_sb = sb("ifo_sb", [B, 3 * HL], f32)  # i [0:128], f [128:256], o [256:384]
    g_sb = sb("g_sb", [B, HL], f32)
    oT_sb = sb("oT_sb", [128, B], f32)
    out_acc = sb("out_acc", [B, S_ * OL], f32)

    ps_gA = [ps("ps_gA0", [B, GA], f32), ps("ps_gA1", [B, GA], f32)]
    ps_gB = [ps("ps_gB0", [B, HL], f32), ps("ps_gB1", [B, HL], f32)]
    ps_out = [ps("ps_out0", [B, OL], f32), ps("ps_out1", [B, OL], f32)]
    ps_oT = ps("ps_oT", [128, B], f32)
    ps_cT = ps("ps_cT", [128, B], f32)

    ones = lambda: misc_sb[0:1, B:B + B]
    outb = lambda: misc_sb[0:1, 2 * B:2 * B + OL]

    OUT_CHUNK = min(32, S_)
    n_chunks = (S_ + OUT_CHUNK - 1) // OUT_CHUNK

    def g_wait(eng, i):
        # wait for exchange i (h_i) fully arrived in gather[i % 4]
        eng.wait_ge(gsem[i % 2], 16 * (i // 2 + 1))

    with nc.Block() as block:

        @block.sync
        def _(sync):
            n = 0
            for dram, buf in ((d_lat, lat_sb), (d_fcw, fcw_sb),
                              (d_fcb, fcb_sb), (d_wc, wc_sb),
                              (d_whh, whh_sb), (d_outw, outw_sb),
                              (d_bias, bias_sb), (d_misc, misc_sb),
                              (d_identf, identf_sb)):
                if n:
                    sync.wait_ge(in_dma, n)
                sync.dma_start(buf[:, :], dram[:, :]).then_inc(in_dma, 16)
                n += 16
            for ch in range(n_chunks):
                hi = min((ch + 1) * OUT_CHUNK, S_)   # steps 1..hi copied
                sync.wait_ge(ocp, hi)
                if ch:
                    sync.wait_ge(odma, 16 * ch)
                sync.dma_start(
                    d_out[:, ch * OUT_CHUNK * OL:hi * OL],
                    out_acc[:, ch * OUT_CHUNK * OL:hi * OL],
                ).then_inc(odma, 16)
            sync.wait_ge(odma, 16 * n_chunks)

        @block.gpsimd
        def _(gp):
            pid = gp.partition_id()
            off = pid * B
            dyn_out = [AP(gather[b_].ap().tensor, off,
                          gather[b_][:, 0:B].ap.copy()) for b_ in range(4)]
            gp.wait_ge(in_dma, 144)
            gp.bir_kernel_barrier_wait([list(range(NC))])
            for i in range(S_ + 1):
                if i:
                    gp.wait_ge(lsem, 16 * i)   # ring: prev send drained
                gp.remote_dma_broadcast(
                    dyn_out[i % 4],
                    hT_sb[i % 2][:, :],
                    remote_sem=gsem[i % 2],
                    local_sem=lsem,
                    rdests=[(0, k) for k in range(NC)],
                ).then_inc(psem, 1)
                gp.wait_ge(psem, i + 1)
                if i == 0:
                    gp.wait_ge(hsem, 1)
                else:
                    gp.wait_ge(dvesem, DVE_H(i))
                gp.trigger_dma(count=1)

        @block.tensor
        def _(te):
            mm = te.matmul

            def dummies(n, bank):
                for _ in range(n):
                    mm(ps_gA[bank][:, :], lat_sb[:, 0:B], wc_sb[:, 0:GA],
                       start=True, stop=True)

            def outproj(j):
                # out_j = h_j @ out_w.T + out_b ; h_j is in gather[j % 4]
                po = ps_out[j % 2]
                if j > 2:
                    te.wait_ge(ocp, j - 2)
                mm(po[:, :], ones(), outb(), start=True, stop=False)
                gb = gather[j % 4]
                for k in range(NC):
                    last = k == NC - 1
                    ins = mm(po[:, :], gb[:, k * B:(k + 1) * B],
                             outw_sb[:, k * OL:(k + 1) * OL],
                             start=False, stop=last)
                    if last:
                        ins.then_inc(osem)           # osem = j

            te.wait_ge(in_dma, 144)
            # h0T = (fc_w @ latent.T) slice; fc_b added on ACT
            mm(ps_cT[:, :], fcw_sb[:, 0:HL], lat_sb[:, 0:B],
               start=True, stop=False)
            mm(ps_cT[:, :], fcw_sb[:, HL:2 * HL], lat_sb[:, B:2 * B],
               start=False, stop=True).then_inc(mma)          # mma = 1

            for t in range(1, S_ + 1):
                W = whh_sb if t == 1 else wc_sb
                pA = ps_gA[t % 2]
                pB = ps_gB[t % 2]
                # bias prefetch into this step's psum banks (during flight)
                if t > 2:
                    te.wait_ge(actsem, ACT_SO(t - 2))   # banks free
                mm(pA[:, :], ones(), bias_sb[0:1, 0:GA],
                   start=True, stop=False)
                mm(pB[:, :], ones(), bias_sb[0:1, GA:GS],
                   start=True, stop=False)
                if t > 1:
                    dummies(dummies_pre, (t + 1) % 2)
                g_wait(te, t - 1)
                gp_buf = gather[(t - 1) % 4]
                for k in range(NC):      # region A: i, f, g
                    last = k == NC - 1
                    ins = mm(pA[:, :], gp_buf[:, k * B:(k + 1) * B],
                             W[:, k * GS:k * GS + GA],
                             start=False, stop=last)
                    if last:
                        ins.then_inc(mma)            # mma = t + 1
                for k in range(NC):      # region B: o
                    last = k == NC - 1
                    ins = mm(pB[:, :], gp_buf[:, k * B:(k + 1) * B],
                             W[:, k * GS + GA:(k + 1) * GS],
                             start=False, stop=last)
                    if last:
                        ins.then_inc(mmb)            # mmb = t
                if t >= 2:
                    outproj(t - 1)
                # transpose o_t -> ps_oT (ACT copies it to SBUF after)
                te.wait_ge(actsem, ACT_SO(t))
                te.transpose(ps_oT[:, :], ifo_sb[:, 2 * HL:3 * HL],
                             identf_sb[:, :]).then_inc(ptsem)   # 2t-1
                dummies(dummies_mid, (t + 1) % 2)
                # transpose tanh(c_t) -> ps_cT (read by DVE hT_{t-1})
                te.wait_ge(actsem, ACT_TC(t))
                if t >= 2:
                    te.wait_ge(dvesem, DVE_H(t - 1))
                te.transpose(ps_cT[:, :], tanhc_sb[:, :],
                             identf_sb[:, :]).then_inc(ptsem)   # 2t
                dummies(dummies_send, (t + 1) % 2)
                te.wait_ge(lsem, 16 * (t + 1))   # exchange t send-complete
                dummies(dummies_flight, (t + 1) % 2)

            g_wait(te, S_)
            outproj(S_)

        @block.scalar
        def _(act):
            act.wait_ge(in_dma, 144)
            act.wait_ge(mma, 1)
            act.activation(hT_sb[0][:, :], ps_cT[:, :], AF.Identity,
                           bias=fcb_sb[:, 0:1]).then_inc(hsem)   # hsem = 1
            for t in range(1, S_ + 1):
                act.wait_ge(mma, t + 1)
                if t >= 2:
                    # i/f cols of ifo freed once DVE t1_{t-1} ran (<= hT)
                    act.wait_ge(dvesem, DVE_H(t - 1))
                act.activation(ifo_sb[:, 0:2 * HL],
                               ps_gA[t % 2][:, 0:2 * HL],
                               AF.Sigmoid).then_inc(actsem)      # 5t-4
                act.activation(g_sb[:, :], ps_gA[t % 2][:, 2 * HL:3 * HL],
                               AF.Tanh).then_inc(actsem)         # 5t-3
                act.wait_ge(mmb, t)
                if t >= 2:
                    act.wait_ge(ptsem, 2 * (t - 1) - 1)  # o WAR vs transp_o
                act.activation(ifo_sb[:, 2 * HL:3 * HL],
                               ps_gB[t % 2][:, :],
                               AF.Sigmoid).then_inc(actsem)      # 5t-2
                act.wait_ge(dvesem, DVE_C(t))
                if t >= 2:
                    act.wait_ge(ptsem, 2 * (t - 1))  # tanhc WAR vs transp_c
                act.activation(tanhc_sb[:, :], c_sb[:, :],
                               AF.Tanh).then_inc(actsem)         # 5t-1
                act.wait_ge(ptsem, 2 * t - 1)        # transpose_o_t done
                act.activation(oT_sb[:, :], ps_oT[:, :],
                               AF.Copy).then_inc(actsem)         # 5t

        @block.vector
        def _(dve):
            tt = dve.tensor_tensor
            dve.wait_ge(in_dma, 144)
            for t in range(1, S_ + 1):
                if t == 1:
                    dve.wait_ge(actsem, ACT_TG(1))
                    tt(c_sb[:, :], ifo_sb[:, 0:HL], g_sb[:, :],
                       ALU.mult).then_inc(dvesem)        # c_1 = i*g
                else:
                    dve.wait_ge(actsem, ACT_SIF(t))
                    tt(c2_sb[:, :], c_sb[:, :], ifo_sb[:, HL:2 * HL],
                       ALU.mult).then_inc(dvesem)        # c2 = c*f
                    dve.wait_ge(actsem, ACT_TG(t))
                    tt(t1_sb[:, :], ifo_sb[:, 0:HL], g_sb[:, :],
                       ALU.mult).then_inc(dvesem)        # t1 = i*g
                    tt(c_sb[:, :], c2_sb[:, :], t1_sb[:, :],
                       ALU.add).then_inc(dvesem)         # c = c2 + t1
                dve.wait_ge(ptsem, 2 * t)
                dve.wait_ge(actsem, ACT_OC(t))    # oT_sb in SBUF
                if t >= 2:
                    dve.wait_ge(lsem, 16 * (t - 1))   # hT buf send-complete
                tt(hT_sb[t % 2][:, :], oT_sb[:, :], ps_cT[:, :],
                   ALU.mult).then_inc(dvesem)            # hT_t
                if t >= 2:
                    j = t - 1
                    dve.wait_ge(osem, j)
                    dve.tensor_copy(out_acc[:, (j - 1) * OL:j * OL],
                                    ps_out[j % 2][:, :]).then_inc(ocp)  # = j
            dve.wait_ge(osem, S_)
            dve.tensor_copy(out_acc[:, (S_ - 1) * OL:S_ * OL],
                            ps_out[S_ % 2][:, :]).then_inc(ocp)  # = S

    ctx.close()
    nc.finalize()
    return nc


def _prep_inputs(latent, fc_w, fc_b, w_ih, w_hh, b_ih, b_hh, out_w, out_b):
    latent = np.asarray(latent, np.float32)
    fc_w = np.asarray(fc_w, np.float32)
    fc_b = np.asarray(fc_b, np.float32)
    w_ih = np.asarray(w_ih, np.float32)
    w_hh = np.asarray(w_hh, np.float32)
    b_ih = np.asarray(b_ih, np.float32)
    b_hh = np.asarray(b_hh, np.float32)
    out_w = np.asarray(out_w, np.float32)
    out_b = np.asarray(out_b, np.float32)

    wc = w_ih + w_hh
    biasc_full = b_ih + b_hh

    latT = np.zeros((128, 2 * B), np.float32)
    for tki in range(2):
        latT[:, tki * B:(tki + 1) * B] = latent[:, tki * 128:(tki + 1) * 128].T

    in_maps = []
    for j in range(NC):
        hsl = slice(HL * j, HL * (j + 1))
        # gate row order per slot: i, f, g, o  (regions A = i,f,g / B = o)
        rows = np.concatenate([
            np.arange(0 * H + HL * j, 0 * H + HL * (j + 1)),   # i
            np.arange(1 * H + HL * j, 1 * H + HL * (j + 1)),   # f
            np.arange(2 * H + HL * j, 2 * H + HL * (j + 1)),   # g
            np.arange(3 * H + HL * j, 3 * H + HL * (j + 1)),   # o
        ])
        wcT = np.zeros((128, NC * GS), np.float32)
        whhT = np.zeros((128, NC * GS), np.float32)
        outwT = np.zeros((128, NC * OL), np.float32)
        for s in range(NC):   # slot s = rank s's 128-row h slice
            ksl = slice(128 * s, 128 * (s + 1))
            wcT[:, s * GS:(s + 1) * GS] = wc[rows][:, ksl].T
            whhT[:, s * GS:(s + 1) * GS] = w_hh[rows][:, ksl].T
            outwT[:, s * OL:(s + 1) * OL] = out_w[OL * j:OL * (j + 1), ksl].T
        fcwT = np.zeros((128, 2 * HL), np.float32)
        for tki in range(2):
            fcwT[:, tki * HL:(tki + 1) * HL] = \
                fc_w[hsl, tki * 128:(tki + 1) * 128].T
        misc = np.zeros((B, 3 * B), np.float32)
        misc[:, 0:B] = np.eye(B)
        misc[0, B:2 * B] = 1.0
        misc[0, 2 * B:2 * B + OL] = out_b[OL * j:OL * (j + 1)]
        in_maps.append({
            "identf": np.eye(B, dtype=np.float32),
            "latT": latT.astype(BF16),
            "fcwT": fcwT.astype(BF16),
            "fcb": fc_b[hsl].reshape(128, 1).astype(np.float32),
            "wcT": wcT.astype(BF16),
            "whhT": whhT.astype(BF16),
            "outwT": outwT.astype(BF16),
            "biasc": biasc_full[rows].reshape(1, GS).astype(BF16),
            "misc": misc.astype(BF16),
        })
    return in_maps


def _install_profile_shim():
    import types
    if 'antenv.axon_hooks' in sys.modules:
        return
    m = types.ModuleType('antenv.axon_hooks')
    m._hook = None
    m.set_axon_ntff_profile_hook = lambda h: setattr(m, '_hook', h)
    m.get_axon_ntff_profile_hook = lambda: m._hook
    sys.modules['antenv.axon_hooks'] = m
    try:
        import antenv
        antenv.axon_hooks = m
        from trn_agent_boot.trn_boot import _ntff_profile_via_ctypes
        m.set_axon_ntff_profile_hook(
            _ntff_profile_via_ctypes('/opt/axon/libaxon_pjrt.so'))
    except Exception:
        pass


_CACHE = {}


def kernel(latent, seq_len, fc_w, fc_b, w_ih, w_hh, b_ih, b_hh, out_w, out_b):
    from concourse import bass_utils

    s_len = int(seq_len)
    assert s_len == S, f"kernel hardcodes seq_len={S}, got {s_len}"

    if os.environ.get("BASS_TRACE"):
        _install_profile_shim()

    if "nc" not in _CACHE:
        _CACHE["nc"] = _build_nc(s_len)
    nc = _CACHE["nc"]

    in_maps = _prep_inputs(latent, fc_w, fc_b, w_ih, w_hh, b_ih, b_hh,
                           out_w, out_b)

    kw = {}
    if os.environ.get("BASS_TRACE"):
        import tempfile
        kw["trace"] = True
        kw["tmpdir"] = tempfile.mkdtemp(prefix="nn_decoder_")
        print(f"[kernel] trace tmpdir: {kw['tmpdir']}")
    res = bass_utils.run_bass_kernel_spmd(
        nc, in_maps, core_ids=list(range(NC)), **kw)
    if getattr(res, "exec_time_ns", None) is not None:
        print(f"[kernel] exec_time_ns: {res.exec_time_ns}")
        _CACHE["exec_time_ns"] = res.exec_time_ns

    outs = [np.asarray(res.results[j]["out"], np.float32).reshape(B, S, OL)
            for j in range(NC)]
    return np.concatenate(outs, axis=2)


# revision 14
# speedup vs baseline: 1.1542x; 1.1542x over previous
"""LSTM decoder (nn_Decoder) on 8 trn2 NeuronCores.

Tensor-parallel over the gate dimension with a remote-DMA-broadcast
all-gather. Each core owns a 128-row slice of h/c and its 512 gate rows
(i,f,g,o x 128). The reference feeds the LSTM output back as both input
and hidden state (x_t = h_t), so steps >= 2 use h @ (w_ih + w_hh).T + b;
step 1 (x0 = 0) uses w_hh alone.

Exchange: each core fires ONE remote_dma_broadcast per step that writes
its hT [128, 64] bf16 tile directly into slot <rank> of every core's
SBUF gather buffer (dynamic out_ap offset = partition_id * 64), bumping
a remote semaphore (+2/sender, +16/round). No DRAM bounce, no ncfw
collective. 4 gather buffers + 2 parity semaphores give the
out-projection (riding one step behind on the PE) a safe read window.

Per step, the gates PSUM is split into region A = {i,f,g} (cols 0:384)
and B = {o} (384:512), each its own accumulation group, so the ACT
sigmoid/tanh start after A's 8 k-matmuls instead of all 16. The h tile
is produced in transposed layout: PE transposes o and tanh(c) into
PSUM, DVE multiplies them straight into the bf16 hT send tile (no
PSUM->SBUF copy on the critical path). Bias matmuls are prefetched into
PSUM during the exchange flight; lsem-gated dummy matmuls keep the PE
pstate high through the flight window.

Semaphore bookkeeping (step t = 1..S):
  mma:  h0 = 1, gates_t region-A stop = t+1.
  mmb:  gates_t region-B stop = t.
  actsem: sig_if_t=5t-4, tanh_g_t=5t-3, sig_o_t=5t-2, tanh_c_t=5t-1,
          oTcopy_t=5t.
  dvesem: t=1: c_1=1, hT_1=2;  t>=2: c2=4t-5, t1=4t-4, c=4t-3, hT=4t-2.
  ptsem: transpose_o_t=2t-1, transpose_tanhc_t=2t.
  hsem: 1 after the h0 tile copy (exchange 0's payload).
  osem/ocp: outproj psum group / out_acc copy for step j -> j.
  lsem: +16 per broadcast send-complete; psem: +1 per prep.
"""
import os
import sys

sys.path.insert(0, "/opt/trn_rl_repo")

import numpy as np
import ml_dtypes

BF16 = ml_dtypes.bfloat16

B = 64          # batch
L = 256         # latent dim
H = 1024        # hidden
O = 512         # output dim
S = 256         # seq len
NC = 8          # cores
HL = H // NC    # 128, per-core h slice
GS = 4 * HL     # 512, per-core gate rows
OL = O // NC    # 64, per-core out slice
GA = 3 * HL     # 384, region A cols (i, f, g)


def ACT_SIF(t):
    return 5 * t - 4


def ACT_TG(t):
    return 5 * t - 3


def ACT_SO(t):
    return 5 * t - 2


def ACT_TC(t):
    return 5 * t - 1


def ACT_OC(t):
    return 5 * t


def DVE_C(t):
    return 1 if t == 1 else 4 * t - 3


def DVE_H(t):
    return 2 if t == 1 else 4 * t - 2


def _build_nc(s_len, dummies_mid=2, dummies_send=6, dummies_flight=5,
              dummies_pre=1, mode="full"):
    from concourse import bass, mybir
    from concourse import bacc
    from concourse.ap import AP
    from contextlib import ExitStack

    S_ = s_len
    nc = bacc.Bacc("TRN2", debug=False)
    f32 = mybir.dt.float32
    bf16 = mybir.dt.bfloat16
    AF = mybir.ActivationFunctionType
    ALU = mybir.AluOpType

    d_lat = nc.dram_tensor("latT", [128, 2 * B], bf16, kind="ExternalInput")
    d_fcw = nc.dram_tensor("fcwT", [128, 2 * HL], bf16, kind="ExternalInput")
    d_fcb = nc.dram_tensor("fcb", [128, 1], f32, kind="ExternalInput")
    d_wc = nc.dram_tensor("wcT", [128, NC * GS], bf16, kind="ExternalInput")
    d_whh = nc.dram_tensor("whhT", [128, NC * GS], bf16, kind="ExternalInput")
    d_outw = nc.dram_tensor("outwT", [128, NC * OL], bf16, kind="ExternalInput")
    d_bias = nc.dram_tensor("biasc", [1, GS], bf16, kind="ExternalInput")
    d_misc = nc.dram_tensor("misc", [B, 3 * B], bf16, kind="ExternalInput")
    d_identf = nc.dram_tensor("identf", [B, B], f32, kind="ExternalInput")
    d_out = nc.dram_tensor("out", [B, S_ * OL], f32, kind="ExternalOutput")

    ctx = ExitStack()
    sem = lambda n: ctx.enter_context(nc.semaphore(n))
    sb = lambda n, sh, dt: ctx.enter_context(nc.sbuf_tensor(n, sh, dt))
    ps = lambda n, sh, dt: ctx.enter_context(nc.psum_tensor(n, sh, dt))

    in_dma = sem("in_dma")     # input loads, 9 x +16 = 144
    gsem = [sem("gsem0"), sem("gsem1")]   # remote arrivals, +16/round/parity
    lsem = sem("lsem")         # broadcast send-complete, +16 each
    psem = sem("psem")         # prep desc-gen done, +1 each
    mma = sem("mma")           # gates region-A group: h0 -> 1, gates_t -> t+1
    mmb = sem("mmb")           # gates region-B group -> t
    osem = sem("osem")         # outproj psum group done -> t
    ocp = sem("ocp")           # outproj copied to out_acc -> t
    actsem = sem("actsem")     # ACT ops, see module docstring
    dvesem = sem("dvesem")     # DVE ops, see module docstring
    hsem = sem("hsem")         # h0 tile ready
    ptsem = sem("ptsem")       # PE transposes -> 2t-1, 2t
    odma = sem("odma")         # out chunks

    lat_sb = sb("lat_sb", [128, 2 * B], bf16)
    fcw_sb = sb("fcw_sb", [128, 2 * HL], bf16)
    fcb_sb = sb("fcb_sb", [128, 1], f32)
    wc_sb = sb("wc_sb", [128, NC * GS], bf16)
    whh_sb = sb("whh_sb", [128, NC * GS], bf16)
    outw_sb = sb("outw_sb", [128, NC * OL], bf16)
    bias_sb = sb("bias_sb", [1, GS], bf16)
    misc_sb = sb("misc_sb", [B, 3 * B], bf16)
    identf_sb = sb("identf_sb", [B, B], f32)
    gather = [sb(f"gather{b_}", [128, NC * B], bf16) for b_ in range(4)]
    hT_sb = [sb("hT0", [128, B], bf16), sb("hT1", [128, B], bf16)]
    c_sb = sb("c_sb", [B, HL], f32)
    c2_sb = sb("c2_sb", [B, HL], f32)
    t1_sb = sb("t1_sb", [B, HL], f32)
    tanhc_sb = sb("tanhc_sb", [B, HL], f32)
    ifo_sb = sb("ifo_sb", [B, 3 * HL], f32)  # i [0:128], f [128:256], o [256:384]
    g_sb = sb("g_sb", [B, HL], f32)
    oT_sb = sb("oT_sb", [128, B], f32)
    out_acc = sb("out_acc", [B, S_ * OL], f32)

    ps_gA = [ps("ps_gA0", [B, GA], f32), ps("ps_gA1", [B, GA], f32)]
    ps_gB = [ps("ps_gB0", [B, HL], f32), ps("ps_gB1", [B, HL], f32)]
    ps_out = [ps("ps_out0", [B, OL], f32), ps("ps_out1", [B, OL], f32)]
    ps_oT = ps("ps_oT", [128, B], f32)
    ps_cT = ps("ps_cT", [128, B], f32)

    ones = lambda: misc_sb[0:1, B:B + B]
    outb = lambda: misc_sb[0:1, 2 * B:2 * B + OL]

    OUT_CHUNK = min(32, S_)
    n_chunks = (S_ + OUT_CHUNK - 1) // OUT_CHUNK

    def g_wait(eng, i):
        # wait for exchange i (h_i) fully arrived in gather[i % 4]
        eng.wait_ge(gsem[i % 2], 16 * (i // 2 + 1))

    with nc.Block() as block:

        @block.sync
        def _(sync):
            n = 0
            for dram, buf in ((d_lat, lat_sb), (d_fcw, fcw_sb),
                              (d_fcb, fcb_sb), (d_wc, wc_sb),
                              (d_whh, whh_sb), (d_outw, outw_sb),
                              (d_bias, bias_sb), (d_misc, misc_sb),
                              (d_identf, identf_sb)):
                if n:
                    sync.wait_ge(in_dma, n)
                sync.dma_start(buf[:, :], dram[:, :]).then_inc(in_dma, 16)
                n += 16
            for ch in range(n_chunks):
                hi = min((ch + 1) * OUT_CHUNK, S_)   # steps 1..hi copied
                sync.wait_ge(ocp, hi)
                if ch:
                    sync.wait_ge(odma, 16 * ch)
                sync.dma_start(
                    d_out[:, ch * OUT_CHUNK * OL:hi * OL],
                    out_acc[:, ch * OUT_CHUNK * OL:hi * OL],
                ).then_inc(odma, 16)
            sync.wait_ge(odma, 16 * n_chunks)

        @block.gpsimd
        def _(gp):
            pid = gp.partition_id()
            off = pid * B
            dyn_out = [AP(gather[b_].ap().tensor, off,
                          gather[b_][:, 0:B].ap.copy()) for b_ in range(4)]
            gp.wait_ge(in_dma, 144)
            gp.bir_kernel_barrier_wait([list(range(NC))])
            for i in range(S_ + 1):
                if i:
                    gp.wait_ge(lsem, 16 * i)   # ring: prev send drained
                gp.remote_dma_broadcast(
                    dyn_out[i % 4],
                    hT_sb[i % 2][:, :],
                    remote_sem=gsem[i % 2],
                    local_sem=lsem,
                    rdests=[(0, k) for k in range(NC)],
                ).then_inc(psem, 1)
                gp.wait_ge(psem, i + 1)
                if i == 0:
                    gp.wait_ge(hsem, 1)
                else:
                    gp.wait_ge(dvesem, DVE_H(i))
                gp.trigger_dma(count=1)

        @block.tensor
        def _(te):
            mm = te.matmul

            def dummies(n, bank):
                for _ in range(n):
                    mm(ps_gA[bank][:, :], lat_sb[:, 0:B], wc_sb[:, 0:GA],
                       start=True, stop=True)

            def outproj(j):
                # out_j = h_j @ out_w.T + out_b ; h_j is in gather[j % 4]
                po = ps_out[j % 2]
                if j > 2:
                    te.wait_ge(ocp, j - 2)
                mm(po[:, :], ones(), outb(), start=True, stop=False)
                gb = gather[j % 4]
                for k in range(NC):
                    last = k == NC - 1
                    ins = mm(po[:, :], gb[:, k * B:(k + 1) * B],
                             outw_sb[:, k * OL:(k + 1) * OL],
                             start=False, stop=last)
                    if last:
                        ins.then_inc(osem)           # osem = j

            te.wait_ge(in_dma, 144)
            # h0T = (fc_w @ latent.T) slice; fc_b added on ACT
            mm(ps_cT[:, :], fcw_sb[:, 0:HL], lat_sb[:, 0:B],
               start=True, stop=False)
            mm(ps_cT[:, :], fcw_sb[:, HL:2 * HL], lat_sb[:, B:2 * B],
               start=False, stop=True).then_inc(mma)          # mma = 1

            for t in range(1, S_ + 1):
                W = whh_sb if t == 1 else wc_sb
                pA = ps_gA[t % 2]
                pB = ps_gB[t % 2]
                # bias prefetch into this step's psum banks (during flight)
                if t > 2:
                    te.wait_ge(actsem, ACT_SO(t - 2))   # banks free
                mm(pA[:, :], ones(), bias_sb[0:1, 0:GA],
                   start=True, stop=False)
                mm(pB[:, :], ones(), bias_sb[0:1, GA:GS],
                   start=True, stop=False)
                if t > 1:
                    dummies(dummies_pre, (t + 1) % 2)
                g_wait(te, t - 1)
                gp_buf = gather[(t - 1) % 4]
                for k in range(NC):      # region A: i, f, g
                    last = k == NC - 1
                    ins = mm(pA[:, :], gp_buf[:, k * B:(k + 1) * B],
                             W[:, k * GS:k * GS + GA],
                             start=False, stop=last)
                    if last:
                        ins.then_inc(mma)            # mma = t + 1
                for k in range(NC):      # region B: o
                    last = k == NC - 1
                    ins = mm(pB[:, :], gp_buf[:, k * B:(k + 1) * B],
                             W[:, k * GS + GA:(k + 1) * GS],
                             start=False, stop=last)
                    if last:
                        ins.then_inc(mmb)            # mmb = t
                if t >= 2:
                    outproj(t - 1)
                # transpose o_t -> ps_oT (ACT copies it to SBUF after)
                te.wait_ge(actsem, ACT_SO(t))
                te.transpose(ps_oT[:, :], ifo_sb[:, 2 * HL:3 * HL],
                             identf_sb[:, :]).then_inc(ptsem)   # 2t-1
                dummies(dummies_mid, (t + 1) % 2)
                # transpose tanh(c_t) -> ps_cT (read by DVE hT_{t-1})
                te.wait_ge(actsem, ACT_TC(t))
                if t >= 2:
                    te.wait_ge(dvesem, DVE_H(t - 1))
                te.transpose(ps_cT[:, :], tanhc_sb[:, :],
                             identf_sb[:, :]).then_inc(ptsem)   # 2t
                dummies(dummies_send, (t + 1) % 2)
                te.wait_ge(lsem, 16 * (t + 1))   # exchange t send-complete
                dummies(dummies_flight, (t + 1) % 2)

            g_wait(te, S_)
            outproj(S_)

        @block.scalar
        def _(act):
            act.wait_ge(in_dma, 144)
            act.wait_ge(mma, 1)
            act.activation(hT_sb[0][:, :], ps_cT[:, :], AF.Identity,
                           bias=fcb_sb[:, 0:1]).then_inc(hsem)   # hsem = 1
            for t in range(1, S_ + 1):
                act.wait_ge(mma, t + 1)
                if t >= 2:
                    # i/f cols of ifo freed once DVE t1_{t-1} ran (<= hT)
                    act.wait_ge(dvesem, DVE_H(t - 1))
                act.activation(ifo_sb[:, 0:2 * HL],
                               ps_gA[t % 2][:, 0:2 * HL],
                               AF.Sigmoid).then_inc(actsem)      # 5t-4
                act.activation(g_sb[:, :], ps_gA[t % 2][:, 2 * HL:3 * HL],
                               AF.Tanh).then_inc(actsem)         # 5t-3
                act.wait_ge(mmb, t)
                if t >= 2:
                    act.wait_ge(ptsem, 2 * (t - 1) - 1)  # o WAR vs transp_o
                act.activation(ifo_sb[:, 2 * HL:3 * HL],
                               ps_gB[t % 2][:, :],
                               AF.Sigmoid).then_inc(actsem)      # 5t-2
                act.wait_ge(dvesem, DVE_C(t))
                if t >= 2:
                    act.wait_ge(ptsem, 2 * (t - 1))  # tanhc WAR vs transp_c
                act.activation(tanhc_sb[:, :], c_sb[:, :],
                               AF.Tanh).then_inc(actsem)         # 5t-1
                act.wait_ge(ptsem, 2 * t - 1)        # transpose_o_t done
                act.activation(oT_sb[:, :], ps_oT[:, :],
                               AF.Copy).then_inc(actsem)         # 5t

        @block.vector
        def _(dve):
            tt = dve.tensor_tensor
            dve.wait_ge(in_dma, 144)
            for t in range(1, S_ + 1):
                if t == 1:
                    dve.wait_ge(actsem, ACT_TG(1))
                    tt(c_sb[:, :], ifo_sb[:, 0:HL], g_sb[:, :],
                       ALU.mult).then_inc(dvesem)        # c_1 = i*g
                else:
                    dve.wait_ge(actsem, ACT_SIF(t))
                    tt(c2_sb[:, :], c_sb[:, :], ifo_sb[:, HL:2 * HL],
                       ALU.mult).then_inc(dvesem)        # c2 = c*f
                    dve.wait_ge(actsem, ACT_TG(t))
                    tt(t1_sb[:, :], ifo_sb[:, 0:HL], g_sb[:, :],
                       ALU.mult).then_inc(dvesem)        # t1 = i*g
                    tt(c_sb[:, :], c2_sb[:, :], t1_sb[:, :],
                       ALU.add).then_inc(dvesem)         # c = c2 + t1
                dve.wait_ge(ptsem, 2 * t)
                dve.wait_ge(actsem, ACT_OC(t))    # oT_sb in SBUF
                if t >= 2:
                    dve.wait_ge(lsem, 16 * (t - 1))   # hT buf send-complete
                tt(hT_sb[t % 2][:, :], oT_sb[:, :], ps_cT[:, :],
                   ALU.mult).then_inc(dvesem)            # hT_t
                if t >= 2:
                    j = t - 1
                    dve.wait_ge(osem, j)
                    dve.tensor_copy(out_acc[:, (j - 1) * OL:j * OL],
                                    ps_out[j % 2][:, :]).then_inc(ocp)  # = j
            dve.wait_ge(osem, S_)
            dve.tensor_copy(out_acc[:, (S_ - 1) * OL:S_ * OL],
                            ps_out[S_ % 2][:, :]).then_inc(ocp)  # = S

    ctx.close()
    nc.finalize()
    return nc


def _prep_inputs(latent, fc_w, fc_b, w_ih, w_hh, b_ih, b_hh, out_w, out_b):
    latent = np.asarray(latent, np.float32)
    fc_w = np.asarray(fc_w, np.float32)
    fc_b = np.asarray(fc_b, np.float32)
    w_ih = np.asarray(w_ih, np.float32)
    w_hh = np.asarray(w_hh, np.float32)
    b_ih = np.asarray(b_ih, np.float32)
    b_hh = np.asarray(b_hh, np.float32)
    out_w = np.asarray(out_w, np.float32)
    out_b = np.asarray(out_b, np.float32)

    wc = w_ih + w_hh
    biasc_full = b_ih + b_hh

    latT = np.zeros((128, 2 * B), np.float32)
    for tki in range(2):
        latT[:, tki * B:(tki + 1) * B] = latent[:, tki * 128:(tki + 1) * 128].T

    in_maps = []
    for j in range(NC):
        hsl = slice(HL * j, HL * (j + 1))
        # gate row order per slot: i, f, g, o  (regions A = i,f,g / B = o)
        rows = np.concatenate([
            np.arange(0 * H + HL * j, 0 * H + HL * (j + 1)),   # i
            np.arange(1 * H + HL * j, 1 * H + HL * (j + 1)),   # f
            np.arange(2 * H + HL * j, 2 * H + HL * (j + 1)),   # g
            np.arange(3 * H + HL * j, 3 * H + HL * (j + 1)),   # o
        ])
        wcT = np.zeros((128, NC * GS), np.float32)
        whhT = np.zeros((128, NC * GS), np.float32)
        outwT = np.zeros((128, NC * OL), np.float32)
        for s in range(NC):   # slot s = rank s's 128-row h slice
            ksl = slice(128 * s, 128 * (s + 1))
            wcT[:, s * GS:(s + 1) * GS] = wc[rows][:, ksl].T
            whhT[:, s * GS:(s + 1) * GS] = w_hh[rows][:, ksl].T
            outwT[:, s * OL:(s + 1) * OL] = out_w[OL * j:OL * (j + 1), ksl].T
        fcwT = np.zeros((128, 2 * HL), np.float32)
        for tki in range(2):
            fcwT[:, tki * HL:(tki + 1) * HL] = \
                fc_w[hsl, tki * 128:(tki + 1) * 128].T
        misc = np.zeros((B, 3 * B), np.float32)
        misc[:, 0:B] = np.eye(B)
        misc[0, B:2 * B] = 1.0
        misc[0, 2 * B:2 * B + OL] = out_b[OL * j:OL * (j + 1)]
        in_maps.append({
            "identf": np.eye(B, dtype=np.float32),
            "latT": latT.astype(BF16),
            "fcwT": fcwT.astype(BF16),
            "fcb": fc_b[hsl].reshape(128, 1).astype(np.float32),
            "wcT": wcT.astype(BF16),
            "whhT": whhT.astype(BF16),
            "outwT": outwT.astype(BF16),
            "biasc": biasc_full[rows].reshape(1, GS).astype(BF16),
            "misc": misc.astype(BF16),
        })
    return in_maps


def _install_profile_shim():
    import types
    if 'antenv.axon_hooks' in sys.modules:
        return
    m = types.ModuleType('antenv.axon_hooks')
    m._hook = None
    m.set_axon_ntff_profile_hook = lambda h: setattr(m, '_hook', h)
    m.get_axon_ntff_profile_hook = lambda: m._hook
    sys.modules['antenv.axon_hooks'] = m
    try:
        import antenv
        antenv.axon_hooks = m
        from trn_agent_boot.trn_boot import _ntff_profile_via_ctypes
        m.set_axon_ntff_profile_hook(
            _ntff_profile_via_ctypes('/opt/axon/libaxon_pjrt.so'))
    except Exception:
        pass


_CACHE = {}


def kernel(latent, seq_len, fc_w, fc_b, w_ih, w_hh, b_ih, b_hh, out_w, out_b):
    from concourse import bass_utils

    s_len = int(seq_len)
    assert s_len == S, f"kernel hardcodes seq_len={S}, got {s_len}"

    if os.environ.get("BASS_TRACE"):
        _install_profile_shim()

    if "nc" not in _CACHE:
        _CACHE["nc"] = _build_nc(s_len)
    nc = _CACHE["nc"]

    in_maps = _prep_inputs(latent, fc_w, fc_b, w_ih, w_hh, b_ih, b_hh,
                           out_w, out_b)

    kw = {}
    if os.environ.get("BASS_TRACE"):
        import tempfile
        kw["trace"] = True
        kw["tmpdir"] = tempfile.mkdtemp(prefix="nn_decoder_")
        print(f"[kernel] trace tmpdir: {kw['tmpdir']}")
    res = bass_utils.run_bass_kernel_spmd(
        nc, in_maps, core_ids=list(range(NC)), **kw)
    if getattr(res, "exec_time_ns", None) is not None:
        print(f"[kernel] exec_time_ns: {res.exec_time_ns}")
        _CACHE["exec_time_ns"] = res.exec_time_ns

    outs = [np.asarray(res.results[j]["out"], np.float32).reshape(B, S, OL)
            for j in range(NC)]
    return np.concatenate(outs, axis=2)


# revision 15
# speedup vs baseline: 1.2343x; 1.0694x over previous
"""LSTM decoder (nn_Decoder) on 8 trn2 NeuronCores.

Tensor-parallel over the gate dimension with a remote-DMA-broadcast
all-gather. Each core owns a 128-row slice of h/c and its 512 gate rows
(i,f,g,o x 128). The reference feeds the LSTM output back as both input
and hidden state (x_t = h_t), so steps >= 2 use h @ (w_ih + w_hh).T + b;
step 1 (x0 = 0) uses w_hh alone.

Exchange: each core fires ONE remote_dma_broadcast per step that writes
its hT [128, 64] bf16 tile directly into slot <rank> of every core's
SBUF gather buffer (dynamic out_ap offset = partition_id * 64), bumping
a remote semaphore (+2/sender, +16/round). No DRAM bounce, no ncfw
collective. 4 gather buffers + 2 parity semaphores give the
out-projection (riding one step behind on the PE) a safe read window.

Per step, the gates PSUM is split into region A = {i,f,g} (cols 0:384)
and B = {o} (384:512), each its own accumulation group, so the ACT
sigmoid/tanh start after A's 8 k-matmuls instead of all 16. The h tile
is produced in transposed layout: PE transposes o and tanh(c) into
PSUM, DVE multiplies them straight into the bf16 hT send tile (no
PSUM->SBUF copy on the critical path). Bias matmuls are prefetched into
PSUM during the exchange flight; lsem-gated dummy matmuls keep the PE
pstate high through the flight window.

Semaphore bookkeeping (step t = 1..S):
  mma:  h0 = 1, gates_t region-A stop = t+1.
  mmb:  gates_t region-B stop = t.
  actsem: sig_if_t=5t-4, tanh_g_t=5t-3, sig_o_t=5t-2, tanh_c_t=5t-1,
          oTcopy_t=5t.
  dvesem: t=1: c_1=1, hT_1=2;  t>=2: c2=4t-5, t1=4t-4, c=4t-3, hT=4t-2.
  ptsem: transpose_o_t=2t-1, transpose_tanhc_t=2t.
  hsem: 1 after the h0 tile copy (exchange 0's payload).
  osem/ocp: outproj psum group / out_acc copy for step j -> j.
  lsem: +16 per broadcast send-complete; psem: +1 per prep.
"""
import os
import sys

sys.path.insert(0, "/opt/trn_rl_repo")

import numpy as np
import ml_dtypes

BF16 = ml_dtypes.bfloat16

B = 64          # batch
L = 256         # latent dim
H = 1024        # hidden
O = 512         # output dim
S = 256         # seq len
NC = 8          # cores
HL = H // NC    # 128, per-core h slice
GS = 4 * HL     # 512, per-core gate rows
OL = O // NC    # 64, per-core out slice
GA = 3 * HL     # 384, region A cols (i, f, g)


def ACT_SIF(t):
    return 5 * t - 4


def ACT_TG(t):
    return 5 * t - 3


def ACT_SO(t):
    return 5 * t - 2


def ACT_TC(t):
    return 5 * t - 1


def ACT_OC(t):
    return 5 * t


def DVE_C(t):
    return 1 if t == 1 else 4 * t - 3


def DVE_H(t):
    return 2 if t == 1 else 4 * t - 2


def _build_nc(s_len, dummies_mid=2, dummies_send=10, dummies_flight=6,
              dummies_pre=1, mode="full"):
    from concourse import bass, mybir
    from concourse import bacc
    from concourse.ap import AP
    from contextlib import ExitStack

    S_ = s_len
    nc = bacc.Bacc("TRN2", debug=False)
    f32 = mybir.dt.float32
    bf16 = mybir.dt.bfloat16
    AF = mybir.ActivationFunctionType
    ALU = mybir.AluOpType

    d_lat = nc.dram_tensor("latT", [128, 2 * B], bf16, kind="ExternalInput")
    d_fcw = nc.dram_tensor("fcwT", [128, 2 * HL], bf16, kind="ExternalInput")
    d_fcb = nc.dram_tensor("fcb", [128, 1], f32, kind="ExternalInput")
    d_wc = nc.dram_tensor("wcT", [128, NC * GS], bf16, kind="ExternalInput")
    d_whh = nc.dram_tensor("whhT", [128, NC * GS], bf16, kind="ExternalInput")
    d_outw = nc.dram_tensor("outwT", [128, NC * OL], bf16, kind="ExternalInput")
    d_bias = nc.dram_tensor("biasc", [1, GS], bf16, kind="ExternalInput")
    d_misc = nc.dram_tensor("misc", [B, 3 * B], bf16, kind="ExternalInput")
    d_identf = nc.dram_tensor("identf", [B, B], f32, kind="ExternalInput")
    d_out = nc.dram_tensor("out", [B, S_ * OL], f32, kind="ExternalOutput")

    ctx = ExitStack()
    sem = lambda n: ctx.enter_context(nc.semaphore(n))
    sb = lambda n, sh, dt: ctx.enter_context(nc.sbuf_tensor(n, sh, dt))
    ps = lambda n, sh, dt: ctx.enter_context(nc.psum_tensor(n, sh, dt))

    in_dma = sem("in_dma")     # input loads, 9 x +16 = 144
    gsem = [sem("gsem0"), sem("gsem1")]   # remote arrivals, +16/round/parity
    lsem = sem("lsem")         # broadcast send-complete, +16 each
    psem = sem("psem")         # prep desc-gen done, +1 each
    mma = sem("mma")           # gates region-A group: h0 -> 1, gates_t -> t+1
    mmb = sem("mmb")           # gates region-B group -> t
    osem = sem("osem")         # outproj psum group done -> t
    ocp = sem("ocp")           # outproj copied to out_acc -> t
    actsem = sem("actsem")     # ACT ops, see module docstring
    dvesem = sem("dvesem")     # DVE ops, see module docstring
    hsem = sem("hsem")         # h0 tile ready
    ptsem = sem("ptsem")       # PE transposes -> 2t-1, 2t
    odma = sem("odma")         # out chunks

    lat_sb = sb("lat_sb", [128, 2 * B], bf16)
    fcw_sb = sb("fcw_sb", [128, 2 * HL], bf16)
    fcb_sb = sb("fcb_sb", [128, 1], f32)
    wc_sb = sb("wc_sb", [128, NC * GS], bf16)
    whh_sb = sb("whh_sb", [128, NC * GS], bf16)
    outw_sb = sb("outw_sb", [128, NC * OL], bf16)
    bias_sb = sb("bias_sb", [1, GS], bf16)
    misc_sb = sb("misc_sb", [B, 3 * B], bf16)
    identf_sb = sb("identf_sb", [B, B], f32)
    gather = [sb(f"gather{b_}", [128, NC * B], bf16) for b_ in range(4)]
    hT_sb = [sb("hT0", [128, B], bf16), sb("hT1", [128, B], bf16)]
    c_sb = sb("c_sb", [B, HL], f32)
    c2_sb = sb("c2_sb", [B, HL], f32)
    t1_sb = sb("t1_sb", [B, HL], f32)
    tanhc_sb = sb("tanhc_sb", [B, HL], f32)
    ifo_sb = sb("ifo_sb", [B, 3 * HL], f32)  # i [0:128], f [128:256], o [256:384]
    g_sb = sb("g_sb", [B, HL], f32)
    oT_sb = sb("oT_sb", [128, B], f32)
    out_acc = sb("out_acc", [B, S_ * OL], f32)

    ps_gA = [ps("ps_gA0", [B, GA], f32), ps("ps_gA1", [B, GA], f32)]
    ps_gB = [ps("ps_gB0", [B, HL], f32), ps("ps_gB1", [B, HL], f32)]
    ps_out = [ps("ps_out0", [B, OL], f32), ps("ps_out1", [B, OL], f32)]
    ps_oT = ps("ps_oT", [128, B], f32)
    ps_cT = ps("ps_cT", [128, B], f32)

    ones = lambda: misc_sb[0:1, B:B + B]
    outb = lambda: misc_sb[0:1, 2 * B:2 * B + OL]

    OUT_CHUNK = min(32, S_)
    n_chunks = (S_ + OUT_CHUNK - 1) // OUT_CHUNK

    def g_wait(eng, i):
        # wait for exchange i (h_i) fully arrived in gather[i % 4]
        eng.wait_ge(gsem[i % 2], 16 * (i // 2 + 1))

    with nc.Block() as block:

        @block.sync
        def _(sync):
            n = 0
            for dram, buf in ((d_lat, lat_sb), (d_fcw, fcw_sb),
                              (d_fcb, fcb_sb), (d_wc, wc_sb),
                              (d_whh, whh_sb), (d_outw, outw_sb),
                              (d_bias, bias_sb), (d_misc, misc_sb),
                              (d_identf, identf_sb)):
                if n:
                    sync.wait_ge(in_dma, n)
                sync.dma_start(buf[:, :], dram[:, :]).then_inc(in_dma, 16)
                n += 16
            for ch in range(n_chunks):
                hi = min((ch + 1) * OUT_CHUNK, S_)   # steps 1..hi copied
                sync.wait_ge(ocp, hi)
                if ch:
                    sync.wait_ge(odma, 16 * ch)
                sync.dma_start(
                    d_out[:, ch * OUT_CHUNK * OL:hi * OL],
                    out_acc[:, ch * OUT_CHUNK * OL:hi * OL],
                ).then_inc(odma, 16)
            sync.wait_ge(odma, 16 * n_chunks)

        @block.gpsimd
        def _(gp):
            pid = gp.partition_id()
            off = pid * B
            dyn_out = [AP(gather[b_].ap().tensor, off,
                          gather[b_][:, 0:B].ap.copy()) for b_ in range(4)]
            gp.wait_ge(in_dma, 144)
            gp.bir_kernel_barrier_wait([list(range(NC))])
            for i in range(S_ + 1):
                if i:
                    gp.wait_ge(lsem, 16 * i)   # ring: prev send drained
                gp.remote_dma_broadcast(
                    dyn_out[i % 4],
                    hT_sb[i % 2][:, :],
                    remote_sem=gsem[i % 2],
                    local_sem=lsem,
                    rdests=[(0, k) for k in range(NC)],
                ).then_inc(psem, 1)
                gp.wait_ge(psem, i + 1)
                if i == 0:
                    gp.wait_ge(hsem, 1)
                else:
                    gp.wait_ge(dvesem, DVE_H(i))
                gp.trigger_dma(count=1)

        @block.tensor
        def _(te):
            mm = te.matmul

            def dummies(n, bank):
                for _ in range(n):
                    mm(ps_gA[bank][:, :], lat_sb[:, 0:B], wc_sb[:, 0:GA],
                       start=True, stop=True)

            def outproj(j):
                # out_j = h_j @ out_w.T + out_b ; h_j is in gather[j % 4]
                po = ps_out[j % 2]
                if j > 2:
                    te.wait_ge(ocp, j - 2)
                mm(po[:, :], ones(), outb(), start=True, stop=False)
                gb = gather[j % 4]
                for k in range(NC):
                    last = k == NC - 1
                    ins = mm(po[:, :], gb[:, k * B:(k + 1) * B],
                             outw_sb[:, k * OL:(k + 1) * OL],
                             start=False, stop=last)
                    if last:
                        ins.then_inc(osem)           # osem = j

            te.wait_ge(in_dma, 144)
            # h0T = (fc_w @ latent.T) slice; fc_b added on ACT
            mm(ps_cT[:, :], fcw_sb[:, 0:HL], lat_sb[:, 0:B],
               start=True, stop=False)
            mm(ps_cT[:, :], fcw_sb[:, HL:2 * HL], lat_sb[:, B:2 * B],
               start=False, stop=True).then_inc(mma)          # mma = 1

            for t in range(1, S_ + 1):
                W = whh_sb if t == 1 else wc_sb
                pA = ps_gA[t % 2]
                pB = ps_gB[t % 2]
                # bias prefetch into this step's psum banks (during flight)
                if t > 2:
                    te.wait_ge(actsem, ACT_SO(t - 2))   # banks free
                mm(pA[:, :], ones(), bias_sb[0:1, 0:GA],
                   start=True, stop=False)
                mm(pB[:, :], ones(), bias_sb[0:1, GA:GS],
                   start=True, stop=False)
                if t > 1:
                    dummies(dummies_pre, (t + 1) % 2)
                g_wait(te, t - 1)
                gp_buf = gather[(t - 1) % 4]
                for k in range(NC):      # region A: i, f, g
                    last = k == NC - 1
                    ins = mm(pA[:, :], gp_buf[:, k * B:(k + 1) * B],
                             W[:, k * GS:k * GS + GA],
                             start=False, stop=last)
                    if last:
                        ins.then_inc(mma)            # mma = t + 1
                for k in range(NC):      # region B: o
                    last = k == NC - 1
                    ins = mm(pB[:, :], gp_buf[:, k * B:(k + 1) * B],
                             W[:, k * GS + GA:(k + 1) * GS],
                             start=False, stop=last)
                    if last:
                        ins.then_inc(mmb)            # mmb = t
                if t >= 2:
                    outproj(t - 1)
                # transpose o_t -> ps_oT (ACT copies it to SBUF after)
                te.wait_ge(actsem, ACT_SO(t))
                te.transpose(ps_oT[:, :], ifo_sb[:, 2 * HL:3 * HL],
                             identf_sb[:, :]).then_inc(ptsem)   # 2t-1
                dummies(dummies_mid, (t + 1) % 2)
                # transpose tanh(c_t) -> ps_cT (read by DVE hT_{t-1})
                te.wait_ge(actsem, ACT_TC(t))
                if t >= 2:
                    te.wait_ge(dvesem, DVE_H(t - 1))
                te.transpose(ps_cT[:, :], tanhc_sb[:, :],
                             identf_sb[:, :]).then_inc(ptsem)   # 2t
                dummies(dummies_send, (t + 1) % 2)
                te.wait_ge(lsem, 16 * (t + 1))   # exchange t send-complete
                dummies(dummies_flight, (t + 1) % 2)

            g_wait(te, S_)
            outproj(S_)

        @block.scalar
        def _(act):
            act.wait_ge(in_dma, 144)
            act.wait_ge(mma, 1)
            act.activation(hT_sb[0][:, :], ps_cT[:, :], AF.Identity,
                           bias=fcb_sb[:, 0:1]).then_inc(hsem)   # hsem = 1
            for t in range(1, S_ + 1):
                act.wait_ge(mma, t + 1)
                if t >= 2:
                    # i/f cols of ifo freed once DVE t1_{t-1} ran (<= hT)
                    act.wait_ge(dvesem, DVE_H(t - 1))
                act.activation(ifo_sb[:, 0:2 * HL],
                               ps_gA[t % 2][:, 0:2 * HL],
                               AF.Sigmoid).then_inc(actsem)      # 5t-4
                act.activation(g_sb[:, :], ps_gA[t % 2][:, 2 * HL:3 * HL],
                               AF.Tanh).then_inc(actsem)         # 5t-3
                act.wait_ge(mmb, t)
                if t >= 2:
                    act.wait_ge(ptsem, 2 * (t - 1) - 1)  # o WAR vs transp_o
                act.activation(ifo_sb[:, 2 * HL:3 * HL],
                               ps_gB[t % 2][:, :],
                               AF.Sigmoid).then_inc(actsem)      # 5t-2
                act.wait_ge(dvesem, DVE_C(t))
                if t >= 2:
                    act.wait_ge(ptsem, 2 * (t - 1))  # tanhc WAR vs transp_c
                act.activation(tanhc_sb[:, :], c_sb[:, :],
                               AF.Tanh).then_inc(actsem)         # 5t-1
                act.wait_ge(ptsem, 2 * t - 1)        # transpose_o_t done
                act.activation(oT_sb[:, :], ps_oT[:, :],
                               AF.Copy).then_inc(actsem)         # 5t

        @block.vector
        def _(dve):
            tt = dve.tensor_tensor
            dve.wait_ge(in_dma, 144)
            for t in range(1, S_ + 1):
                if t == 1:
                    dve.wait_ge(actsem, ACT_TG(1))
                    tt(c_sb[:, :], ifo_sb[:, 0:HL], g_sb[:, :],
                       ALU.mult).then_inc(dvesem)        # c_1 = i*g
                else:
                    dve.wait_ge(actsem, ACT_SIF(t))
                    tt(c2_sb[:, :], c_sb[:, :], ifo_sb[:, HL:2 * HL],
                       ALU.mult).then_inc(dvesem)        # c2 = c*f
                    dve.wait_ge(actsem, ACT_TG(t))
                    tt(t1_sb[:, :], ifo_sb[:, 0:HL], g_sb[:, :],
                       ALU.mult).then_inc(dvesem)        # t1 = i*g
                    tt(c_sb[:, :], c2_sb[:, :], t1_sb[:, :],
                       ALU.add).then_inc(dvesem)         # c = c2 + t1
                dve.wait_ge(ptsem, 2 * t)
                dve.wait_ge(actsem, ACT_OC(t))    # oT_sb in SBUF
                if t >= 2:
                    dve.wait_ge(lsem, 16 * (t - 1))   # hT buf send-complete
                tt(hT_sb[t % 2][:, :], oT_sb[:, :], ps_cT[:, :],
                   ALU.mult).then_inc(dvesem)            # hT_t
                if t >= 2:
                    j = t - 1
                    dve.wait_ge(osem, j)
                    dve.tensor_copy(out_acc[:, (j - 1) * OL:j * OL],
                                    ps_out[j % 2][:, :]).then_inc(ocp)  # = j
            dve.wait_ge(osem, S_)
            dve.tensor_copy(out_acc[:, (S_ - 1) * OL:S_ * OL],
                            ps_out[S_ % 2][:, :]).then_inc(ocp)  # = S

    ctx.close()
    nc.finalize()
    return nc


def _prep_inputs(latent, fc_w, fc_b, w_ih, w_hh, b_ih, b_hh, out_w, out_b):
    latent = np.asarray(latent, np.float32)
    fc_w = np.asarray(fc_w, np.float32)
    fc_b = np.asarray(fc_b, np.float32)
    w_ih = np.asarray(w_ih, np.float32)
    w_hh = np.asarray(w_hh, np.float32)
    b_ih = np.asarray(b_ih, np.float32)
    b_hh = np.asarray(b_hh, np.float32)
    out_w = np.asarray(out_w, np.float32)
    out_b = np.asarray(out_b, np.float32)

    wc = w_ih + w_hh
    biasc_full = b_ih + b_hh

    latT = np.zeros((128, 2 * B), np.float32)
    for tki in range(2):
        latT[:, tki * B:(tki + 1) * B] = latent[:, tki * 128:(tki + 1) * 128].T

    in_maps = []
    for j in range(NC):
        hsl = slice(HL * j, HL * (j + 1))
        # gate row order per slot: i, f, g, o  (regions A = i,f,g / B = o)
        rows = np.concatenate([
            np.arange(0 * H + HL * j, 0 * H + HL * (j + 1)),   # i
            np.arange(1 * H + HL * j, 1 * H + HL * (j + 1)),   # f
            np.arange(2 * H + HL * j, 2 * H + HL * (j + 1)),   # g
            np.arange(3 * H + HL * j, 3 * H + HL * (j + 1)),   # o
        ])
        wcT = np.zeros((128, NC * GS), np.float32)
        whhT = np.zeros((128, NC * GS), np.float32)
        outwT = np.zeros((128, NC * OL), np.float32)
        for s in range(NC):   # slot s = rank s's 128-row h slice
            ksl = slice(128 * s, 128 * (s + 1))
            wcT[:, s * GS:(s + 1) * GS] = wc[rows][:, ksl].T
            whhT[:, s * GS:(s + 1) * GS] = w_hh[rows][:, ksl].T
            outwT[:, s * OL:(s + 1) * OL] = out_w[OL * j:OL * (j + 1), ksl].T
        fcwT = np.zeros((128, 2 * HL), np.float32)
        for tki in range(2):
            fcwT[:, tki * HL:(tki + 1) * HL] = \
                fc_w[hsl, tki * 128:(tki + 1) * 128].T
        misc = np.zeros((B, 3 * B), np.float32)
        misc[:, 0:B] = np.eye(B)
        misc[0, B:2 * B] = 1.0
        misc[0, 2 * B:2 * B + OL] = out_b[OL * j:OL * (j + 1)]
        in_maps.append({
            "identf": np.eye(B, dtype=np.float32),
            "latT": latT.astype(BF16),
            "fcwT": fcwT.astype(BF16),
            "fcb": fc_b[hsl].reshape(128, 1).astype(np.float32),
            "wcT": wcT.astype(BF16),
            "whhT": whhT.astype(BF16),
            "outwT": outwT.astype(BF16),
            "biasc": biasc_full[rows].reshape(1, GS).astype(BF16),
            "misc": misc.astype(BF16),
        })
    return in_maps


def _install_profile_shim():
    import types
    if 'antenv.axon_hooks' in sys.modules:
        return
    m = types.ModuleType('antenv.axon_hooks')
    m._hook = None
    m.set_axon_ntff_profile_hook = lambda h: setattr(m, '_hook', h)
    m.get_axon_ntff_profile_hook = lambda: m._hook
    sys.modules['antenv.axon_hooks'] = m
    try:
        import antenv
        antenv.axon_hooks = m
        from trn_agent_boot.trn_boot import _ntff_profile_via_ctypes
        m.set_axon_ntff_profile_hook(
            _ntff_profile_via_ctypes('/opt/axon/libaxon_pjrt.so'))
    except Exception:
        pass


_CACHE = {}


def kernel(latent, seq_len, fc_w, fc_b, w_ih, w_hh, b_ih, b_hh, out_w, out_b):
    from concourse import bass_utils

    s_len = int(seq_len)
    assert s_len == S, f"kernel hardcodes seq_len={S}, got {s_len}"

    if os.environ.get("BASS_TRACE"):
        _install_profile_shim()

    if "nc" not in _CACHE:
        _CACHE["nc"] = _build_nc(s_len)
    nc = _CACHE["nc"]

    in_maps = _prep_inputs(latent, fc_w, fc_b, w_ih, w_hh, b_ih, b_hh,
                           out_w, out_b)

    kw = {}
    if os.environ.get("BASS_TRACE"):
        import tempfile
        kw["trace"] = True
        kw["tmpdir"] = tempfile.mkdtemp(prefix="nn_decoder_")
        print(f"[kernel] trace tmpdir: {kw['tmpdir']}")
    res = bass_utils.run_bass_kernel_spmd(
        nc, in_maps, core_ids=list(range(NC)), **kw)
    if getattr(res, "exec_time_ns", None) is not None:
        print(f"[kernel] exec_time_ns: {res.exec_time_ns}")
        _CACHE["exec_time_ns"] = res.exec_time_ns

    outs = [np.asarray(res.results[j]["out"], np.float32).reshape(B, S, OL)
            for j in range(NC)]
    return np.concatenate(outs, axis=2)


# revision 16
# speedup vs baseline: 1.2383x; 1.0032x over previous
"""LSTM decoder (nn_Decoder) on 8 trn2 NeuronCores.

Tensor-parallel over the gate dimension with a remote-DMA-broadcast
all-gather. Each core owns a 128-row slice of h/c and its 512 gate rows
(i,f,g,o x 128). The reference feeds the LSTM output back as both input
and hidden state (x_t = h_t), so steps >= 2 use h @ (w_ih + w_hh).T + b;
step 1 (x0 = 0) uses w_hh alone.

Exchange: each core fires ONE remote_dma_broadcast per step that writes
its hT [128, 64] bf16 tile directly into slot <rank> of every core's
SBUF gather buffer (dynamic out_ap offset = partition_id * 64), bumping
a remote semaphore (+2/sender, +16/round). No DRAM bounce, no ncfw
collective. 4 gather buffers + 2 parity semaphores give the
out-projection (riding one step behind on the PE) a safe read window.

Per step, the gates PSUM is split into region A = {i,f,g} (cols 0:384)
and B = {o} (384:512), each its own accumulation group, so the ACT
sigmoid/tanh start after A's 8 k-matmuls instead of all 16. The h tile
is produced in transposed layout: PE transposes o and tanh(c) into
PSUM, DVE multiplies them straight into the bf16 hT send tile (no
PSUM->SBUF copy on the critical path). Bias matmuls are prefetched into
PSUM during the exchange flight; lsem-gated dummy matmuls keep the PE
pstate high through the flight window.

Semaphore bookkeeping (step t = 1..S):
  mma:  h0 = 1, gates_t region-A stop = t+1.
  mmb:  gates_t region-B stop = t.
  actsem: sig_if_t=5t-4, tanh_g_t=5t-3, sig_o_t=5t-2, tanh_c_t=5t-1,
          oTcopy_t=5t.
  dvesem: t=1: c_1=1, hT_1=2;  t>=2: c2=4t-5, t1=4t-4, c=4t-3, hT=4t-2.
  ptsem: transpose_o_t=2t-1, transpose_tanhc_t=2t.
  hsem: 1 after the h0 tile copy (exchange 0's payload).
  osem/ocp: outproj psum group / out_acc copy for step j -> j.
  lsem: +16 per broadcast send-complete; psem: +1 per prep.
"""
import os
import sys

sys.path.insert(0, "/opt/trn_rl_repo")

import numpy as np
import ml_dtypes

BF16 = ml_dtypes.bfloat16

B = 64          # batch
L = 256         # latent dim
H = 1024        # hidden
O = 512         # output dim
S = 256         # seq len
NC = 8          # cores
HL = H // NC    # 128, per-core h slice
GS = 4 * HL     # 512, per-core gate rows
OL = O // NC    # 64, per-core out slice
GA = 3 * HL     # 384, region A cols (i, f, g)


def ACT_SIF(t):
    return 5 * t - 4


def ACT_TG(t):
    return 5 * t - 3


def ACT_SO(t):
    return 5 * t - 2


def ACT_TC(t):
    return 5 * t - 1


def ACT_OC(t):
    return 5 * t


def DVE_C(t):
    return 1 if t == 1 else 4 * t - 3


def DVE_H(t):
    return 2 if t == 1 else 4 * t - 2


def _build_nc(s_len, dummies_mid=2, dummies_send=13, dummies_flight=8,
              dummies_pre=1, mode="full"):
    from concourse import bass, mybir
    from concourse import bacc
    from concourse.ap import AP
    from contextlib import ExitStack

    S_ = s_len
    nc = bacc.Bacc("TRN2", debug=False)
    f32 = mybir.dt.float32
    bf16 = mybir.dt.bfloat16
    AF = mybir.ActivationFunctionType
    ALU = mybir.AluOpType

    d_lat = nc.dram_tensor("latT", [128, 2 * B], bf16, kind="ExternalInput")
    d_fcw = nc.dram_tensor("fcwT", [128, 2 * HL], bf16, kind="ExternalInput")
    d_fcb = nc.dram_tensor("fcb", [128, 1], f32, kind="ExternalInput")
    d_wc = nc.dram_tensor("wcT", [128, NC * GS], bf16, kind="ExternalInput")
    d_whh = nc.dram_tensor("whhT", [128, NC * GS], bf16, kind="ExternalInput")
    d_outw = nc.dram_tensor("outwT", [128, NC * OL], bf16, kind="ExternalInput")
    d_bias = nc.dram_tensor("biasc", [1, GS], bf16, kind="ExternalInput")
    d_misc = nc.dram_tensor("misc", [B, 3 * B], bf16, kind="ExternalInput")
    d_identf = nc.dram_tensor("identf", [B, B], f32, kind="ExternalInput")
    d_out = nc.dram_tensor("out", [B, S_ * OL], f32, kind="ExternalOutput")

    ctx = ExitStack()
    sem = lambda n: ctx.enter_context(nc.semaphore(n))
    sb = lambda n, sh, dt: ctx.enter_context(nc.sbuf_tensor(n, sh, dt))
    ps = lambda n, sh, dt: ctx.enter_context(nc.psum_tensor(n, sh, dt))

    in_dma = sem("in_dma")     # input loads, 9 x +16 = 144
    gsem = [sem("gsem0"), sem("gsem1")]   # remote arrivals, +16/round/parity
    lsem = sem("lsem")         # broadcast send-complete, +16 each
    psem = sem("psem")         # prep desc-gen done, +1 each
    mma = sem("mma")           # gates region-A group: h0 -> 1, gates_t -> t+1
    mmb = sem("mmb")           # gates region-B group -> t
    osem = sem("osem")         # outproj psum group done -> t
    ocp = sem("ocp")           # outproj copied to out_acc -> t
    actsem = sem("actsem")     # ACT ops, see module docstring
    dvesem = sem("dvesem")     # DVE ops, see module docstring
    hsem = sem("hsem")         # h0 tile ready
    ptsem = sem("ptsem")       # PE transposes -> 2t-1, 2t
    odma = sem("odma")         # out chunks

    lat_sb = sb("lat_sb", [128, 2 * B], bf16)
    fcw_sb = sb("fcw_sb", [128, 2 * HL], bf16)
    fcb_sb = sb("fcb_sb", [128, 1], f32)
    wc_sb = sb("wc_sb", [128, NC * GS], bf16)
    whh_sb = sb("whh_sb", [128, NC * GS], bf16)
    outw_sb = sb("outw_sb", [128, NC * OL], bf16)
    bias_sb = sb("bias_sb", [1, GS], bf16)
    misc_sb = sb("misc_sb", [B, 3 * B], bf16)
    identf_sb = sb("identf_sb", [B, B], f32)
    gather = [sb(f"gather{b_}", [128, NC * B], bf16) for b_ in range(4)]
    hT_sb = [sb("hT0", [128, B], bf16), sb("hT1", [128, B], bf16)]
    c_sb = sb("c_sb", [B, HL], f32)
    c2_sb = sb("c2_sb", [B, HL], f32)
    t1_sb = sb("t1_sb", [B, HL], f32)
    tanhc_sb = sb("tanhc_sb", [B, HL], f32)
    ifo_sb = sb("ifo_sb", [B, 3 * HL], f32)  # i [0:128], f [128:256], o [256:384]
    g_sb = sb("g_sb", [B, HL], f32)
    oT_sb = sb("oT_sb", [128, B], f32)
    out_acc = sb("out_acc", [B, S_ * OL], f32)

    ps_gA = [ps("ps_gA0", [B, GA], f32), ps("ps_gA1", [B, GA], f32)]
    ps_gB = [ps("ps_gB0", [B, HL], f32), ps("ps_gB1", [B, HL], f32)]
    ps_out = [ps("ps_out0", [B, OL], f32), ps("ps_out1", [B, OL], f32)]
    ps_oT = ps("ps_oT", [128, B], f32)
    ps_cT = ps("ps_cT", [128, B], f32)

    ones = lambda: misc_sb[0:1, B:B + B]
    outb = lambda: misc_sb[0:1, 2 * B:2 * B + OL]

    OUT_CHUNK = min(32, S_)
    n_chunks = (S_ + OUT_CHUNK - 1) // OUT_CHUNK

    def g_wait(eng, i):
        # wait for exchange i (h_i) fully arrived in gather[i % 4]
        eng.wait_ge(gsem[i % 2], 16 * (i // 2 + 1))

    with nc.Block() as block:

        @block.sync
        def _(sync):
            n = 0
            for dram, buf in ((d_lat, lat_sb), (d_fcw, fcw_sb),
                              (d_fcb, fcb_sb), (d_wc, wc_sb),
                              (d_whh, whh_sb), (d_outw, outw_sb),
                              (d_bias, bias_sb), (d_misc, misc_sb),
                              (d_identf, identf_sb)):
                if n:
                    sync.wait_ge(in_dma, n)
                sync.dma_start(buf[:, :], dram[:, :]).then_inc(in_dma, 16)
                n += 16
            for ch in range(n_chunks):
                hi = min((ch + 1) * OUT_CHUNK, S_)   # steps 1..hi copied
                sync.wait_ge(ocp, hi)
                if ch:
                    sync.wait_ge(odma, 16 * ch)
                sync.dma_start(
                    d_out[:, ch * OUT_CHUNK * OL:hi * OL],
                    out_acc[:, ch * OUT_CHUNK * OL:hi * OL],
                ).then_inc(odma, 16)
            sync.wait_ge(odma, 16 * n_chunks)

        @block.gpsimd
        def _(gp):
            pid = gp.partition_id()
            off = pid * B
            dyn_out = [AP(gather[b_].ap().tensor, off,
                          gather[b_][:, 0:B].ap.copy()) for b_ in range(4)]
            gp.wait_ge(in_dma, 144)
            gp.bir_kernel_barrier_wait([list(range(NC))])
            for i in range(S_ + 1):
                if i:
                    gp.wait_ge(lsem, 16 * i)   # ring: prev send drained
                gp.remote_dma_broadcast(
                    dyn_out[i % 4],
                    hT_sb[i % 2][:, :],
                    remote_sem=gsem[i % 2],
                    local_sem=lsem,
                    rdests=[(0, k) for k in range(NC)],
                ).then_inc(psem, 1)
                gp.wait_ge(psem, i + 1)
                if i == 0:
                    gp.wait_ge(hsem, 1)
                else:
                    gp.wait_ge(dvesem, DVE_H(i))
                gp.trigger_dma(count=1)

        @block.tensor
        def _(te):
            mm = te.matmul

            def dummies(n, bank):
                for _ in range(n):
                    mm(ps_gA[bank][:, :], lat_sb[:, 0:B], wc_sb[:, 0:GA],
                       start=True, stop=True)

            def outproj(j):
                # out_j = h_j @ out_w.T + out_b ; h_j is in gather[j % 4]
                po = ps_out[j % 2]
                if j > 2:
                    te.wait_ge(ocp, j - 2)
                mm(po[:, :], ones(), outb(), start=True, stop=False)
                gb = gather[j % 4]
                for k in range(NC):
                    last = k == NC - 1
                    ins = mm(po[:, :], gb[:, k * B:(k + 1) * B],
                             outw_sb[:, k * OL:(k + 1) * OL],
                             start=False, stop=last)
                    if last:
                        ins.then_inc(osem)           # osem = j

            te.wait_ge(in_dma, 144)
            # h0T = (fc_w @ latent.T) slice; fc_b added on ACT
            mm(ps_cT[:, :], fcw_sb[:, 0:HL], lat_sb[:, 0:B],
               start=True, stop=False)
            mm(ps_cT[:, :], fcw_sb[:, HL:2 * HL], lat_sb[:, B:2 * B],
               start=False, stop=True).then_inc(mma)          # mma = 1

            for t in range(1, S_ + 1):
                W = whh_sb if t == 1 else wc_sb
                pA = ps_gA[t % 2]
                pB = ps_gB[t % 2]
                # bias prefetch into this step's psum banks (during flight)
                if t > 2:
                    te.wait_ge(actsem, ACT_SO(t - 2))   # banks free
                mm(pA[:, :], ones(), bias_sb[0:1, 0:GA],
                   start=True, stop=False)
                mm(pB[:, :], ones(), bias_sb[0:1, GA:GS],
                   start=True, stop=False)
                if t > 1:
                    dummies(dummies_pre, (t + 1) % 2)
                g_wait(te, t - 1)
                gp_buf = gather[(t - 1) % 4]
                for k in range(NC):      # region A: i, f, g
                    last = k == NC - 1
                    ins = mm(pA[:, :], gp_buf[:, k * B:(k + 1) * B],
                             W[:, k * GS:k * GS + GA],
                             start=False, stop=last)
                    if last:
                        ins.then_inc(mma)            # mma = t + 1
                for k in range(NC):      # region B: o
                    last = k == NC - 1
                    ins = mm(pB[:, :], gp_buf[:, k * B:(k + 1) * B],
                             W[:, k * GS + GA:(k + 1) * GS],
                             start=False, stop=last)
                    if last:
                        ins.then_inc(mmb)            # mmb = t
                if t >= 2:
                    outproj(t - 1)
                # transpose o_t -> ps_oT (ACT copies it to SBUF after)
                te.wait_ge(actsem, ACT_SO(t))
                te.transpose(ps_oT[:, :], ifo_sb[:, 2 * HL:3 * HL],
                             identf_sb[:, :]).then_inc(ptsem)   # 2t-1
                dummies(dummies_mid, (t + 1) % 2)
                # transpose tanh(c_t) -> ps_cT (read by DVE hT_{t-1})
                te.wait_ge(actsem, ACT_TC(t))
                if t >= 2:
                    te.wait_ge(dvesem, DVE_H(t - 1))
                te.transpose(ps_cT[:, :], tanhc_sb[:, :],
                             identf_sb[:, :]).then_inc(ptsem)   # 2t
                dummies(dummies_send, (t + 1) % 2)
                te.wait_ge(lsem, 16 * (t + 1))   # exchange t send-complete
                dummies(dummies_flight, (t + 1) % 2)

            g_wait(te, S_)
            outproj(S_)

        @block.scalar
        def _(act):
            act.wait_ge(in_dma, 144)
            act.wait_ge(mma, 1)
            act.activation(hT_sb[0][:, :], ps_cT[:, :], AF.Identity,
                           bias=fcb_sb[:, 0:1]).then_inc(hsem)   # hsem = 1
            for t in range(1, S_ + 1):
                act.wait_ge(mma, t + 1)
                if t >= 2:
                    # i/f cols of ifo freed once DVE t1_{t-1} ran (<= hT)
                    act.wait_ge(dvesem, DVE_H(t - 1))
                act.activation(ifo_sb[:, 0:2 * HL],
                               ps_gA[t % 2][:, 0:2 * HL],
                               AF.Sigmoid).then_inc(actsem)      # 5t-4
                act.activation(g_sb[:, :], ps_gA[t % 2][:, 2 * HL:3 * HL],
                               AF.Tanh).then_inc(actsem)         # 5t-3
                act.wait_ge(mmb, t)
                if t >= 2:
                    act.wait_ge(ptsem, 2 * (t - 1) - 1)  # o WAR vs transp_o
                act.activation(ifo_sb[:, 2 * HL:3 * HL],
                               ps_gB[t % 2][:, :],
                               AF.Sigmoid).then_inc(actsem)      # 5t-2
                act.wait_ge(dvesem, DVE_C(t))
                if t >= 2:
                    act.wait_ge(ptsem, 2 * (t - 1))  # tanhc WAR vs transp_c
                act.activation(tanhc_sb[:, :], c_sb[:, :],
                               AF.Tanh).then_inc(actsem)         # 5t-1
                act.wait_ge(ptsem, 2 * t - 1)        # transpose_o_t done
                act.activation(oT_sb[:, :], ps_oT[:, :],
                               AF.Copy).then_inc(actsem)         # 5t

        @block.vector
        def _(dve):
            tt = dve.tensor_tensor
            dve.wait_ge(in_dma, 144)
            for t in range(1, S_ + 1):
                if t == 1:
                    dve.wait_ge(actsem, ACT_TG(1))
                    tt(c_sb[:, :], ifo_sb[:, 0:HL], g_sb[:, :],
                       ALU.mult).then_inc(dvesem)        # c_1 = i*g
                else:
                    dve.wait_ge(actsem, ACT_SIF(t))
                    tt(c2_sb[:, :], c_sb[:, :], ifo_sb[:, HL:2 * HL],
                       ALU.mult).then_inc(dvesem)        # c2 = c*f
                    dve.wait_ge(actsem, ACT_TG(t))
                    tt(t1_sb[:, :], ifo_sb[:, 0:HL], g_sb[:, :],
                       ALU.mult).then_inc(dvesem)        # t1 = i*g
                    tt(c_sb[:, :], c2_sb[:, :], t1_sb[:, :],
                       ALU.add).then_inc(dvesem)         # c = c2 + t1
                dve.wait_ge(ptsem, 2 * t)
                dve.wait_ge(actsem, ACT_OC(t))    # oT_sb in SBUF
                if t >= 2:
                    dve.wait_ge(lsem, 16 * (t - 1))   # hT buf send-complete
                tt(hT_sb[t % 2][:, :], oT_sb[:, :], ps_cT[:, :],
                   ALU.mult).then_inc(dvesem)            # hT_t
                if t >= 2:
                    j = t - 1
                    dve.wait_ge(osem, j)
                    dve.tensor_copy(out_acc[:, (j - 1) * OL:j * OL],
                                    ps_out[j % 2][:, :]).then_inc(ocp)  # = j
            dve.wait_ge(osem, S_)
            dve.tensor_copy(out_acc[:, (S_ - 1) * OL:S_ * OL],
                            ps_out[S_ % 2][:, :]).then_inc(ocp)  # = S

    ctx.close()
    nc.finalize()
    return nc


def _prep_inputs(latent, fc_w, fc_b, w_ih, w_hh, b_ih, b_hh, out_w, out_b):
    latent = np.asarray(latent, np.float32)
    fc_w = np.asarray(fc_w, np.float32)
    fc_b = np.asarray(fc_b, np.float32)
    w_ih = np.asarray(w_ih, np.float32)
    w_hh = np.asarray(w_hh, np.float32)
    b_ih = np.asarray(b_ih, np.float32)
    b_hh = np.asarray(b_hh, np.float32)
    out_w = np.asarray(out_w, np.float32)
    out_b = np.asarray(out_b, np.float32)

    wc = w_ih + w_hh
    biasc_full = b_ih + b_hh

    latT = np.zeros((128, 2 * B), np.float32)
    for tki in range(2):
        latT[:, tki * B:(tki + 1) * B] = latent[:, tki * 128:(tki + 1) * 128].T

    in_maps = []
    for j in range(NC):
        hsl = slice(HL * j, HL * (j + 1))
        # gate row order per slot: i, f, g, o  (regions A = i,f,g / B = o)
        rows = np.concatenate([
            np.arange(0 * H + HL * j, 0 * H + HL * (j + 1)),   # i
            np.arange(1 * H + HL * j, 1 * H + HL * (j + 1)),   # f
            np.arange(2 * H + HL * j, 2 * H + HL * (j + 1)),   # g
            np.arange(3 * H + HL * j, 3 * H + HL * (j + 1)),   # o
        ])
        wcT = np.zeros((128, NC * GS), np.float32)
        whhT = np.zeros((128, NC * GS), np.float32)
        outwT = np.zeros((128, NC * OL), np.float32)
        for s in range(NC):   # slot s = rank s's 128-row h slice
            ksl = slice(128 * s, 128 * (s + 1))
            wcT[:, s * GS:(s + 1) * GS] = wc[rows][:, ksl].T
            whhT[:, s * GS:(s + 1) * GS] = w_hh[rows][:, ksl].T
            outwT[:, s * OL:(s + 1) * OL] = out_w[OL * j:OL * (j + 1), ksl].T
        fcwT = np.zeros((128, 2 * HL), np.float32)
        for tki in range(2):
            fcwT[:, tki * HL:(tki + 1) * HL] = \
                fc_w[hsl, tki * 128:(tki + 1) * 128].T
        misc = np.zeros((B, 3 * B), np.float32)
        misc[:, 0:B] = np.eye(B)
        misc[0, B:2 * B] = 1.0
        misc[0, 2 * B:2 * B + OL] = out_b[OL * j:OL * (j + 1)]
        in_maps.append({
            "identf": np.eye(B, dtype=np.float32),
            "latT": latT.astype(BF16),
            "fcwT": fcwT.astype(BF16),
            "fcb": fc_b[hsl].reshape(128, 1).astype(np.float32),
            "wcT": wcT.astype(BF16),
            "whhT": whhT.astype(BF16),
            "outwT": outwT.astype(BF16),
            "biasc": biasc_full[rows].reshape(1, GS).astype(BF16),
            "misc": misc.astype(BF16),
        })
    return in_maps


def _install_profile_shim():
    import types
    if 'antenv.axon_hooks' in sys.modules:
        return
    m = types.ModuleType('antenv.axon_hooks')
    m._hook = None
    m.set_axon_ntff_profile_hook = lambda h: setattr(m, '_hook', h)
    m.get_axon_ntff_profile_hook = lambda: m._hook
    sys.modules['antenv.axon_hooks'] = m
    try:
        import antenv
        antenv.axon_hooks = m
        from trn_agent_boot.trn_boot import _ntff_profile_via_ctypes
        m.set_axon_ntff_profile_hook(
            _ntff_profile_via_ctypes('/opt/axon/libaxon_pjrt.so'))
    except Exception:
        pass


_CACHE = {}


def kernel(latent, seq_len, fc_w, fc_b, w_ih, w_hh, b_ih, b_hh, out_w, out_b):
    from concourse import bass_utils

    s_len = int(seq_len)
    assert s_len == S, f"kernel hardcodes seq_len={S}, got {s_len}"

    if os.environ.get("BASS_TRACE"):
        _install_profile_shim()

    if "nc" not in _CACHE:
        _CACHE["nc"] = _build_nc(s_len)
    nc = _CACHE["nc"]

    in_maps = _prep_inputs(latent, fc_w, fc_b, w_ih, w_hh, b_ih, b_hh,
                           out_w, out_b)

    kw = {}
    if os.environ.get("BASS_TRACE"):
        import tempfile
        kw["trace"] = True
        kw["tmpdir"] = tempfile.mkdtemp(prefix="nn_decoder_")
        print(f"[kernel] trace tmpdir: {kw['tmpdir']}")
    res = bass_utils.run_bass_kernel_spmd(
        nc, in_maps, core_ids=list(range(NC)), **kw)
    if getattr(res, "exec_time_ns", None) is not None:
        print(f"[kernel] exec_time_ns: {res.exec_time_ns}")
        _CACHE["exec_time_ns"] = res.exec_time_ns

    outs = [np.asarray(res.results[j]["out"], np.float32).reshape(B, S, OL)
            for j in range(NC)]
    return np.concatenate(outs, axis=2)
